# revision 5
# baseline (speedup 1.0000x reference)
"""ANIMAZero recurrent cell on 8 TRN2 NeuronCores (Bass/Tile).

Data-parallel: batch 1024 is split into 8 shards of 128; each core runs
the full T=256 recurrence on its shard. Per step, the three D=32 states
[W; I; A] live stacked on SBUF partitions so each gate group is one
fp16 matmul; sigmoid/tanh run on ScalarE with fused per-partition
biases; elementwise gating runs on VectorE in fp16 2x mode. The phi
output projection accumulates 4 steps in a PSUM bank and is evacuated
on ScalarE into the per-step idle window.
"""

import os
import sys

sys.path.insert(0, "/opt/trn_rl_repo")
import numpy as np
import bass_rust
import concourse.bass as bass
import concourse.tile as tile
from concourse import mybir

F32 = mybir.dt.float32
F16 = mybir.dt.float16
SIG = mybir.ActivationFunctionType.Sigmoid
TANH = mybir.ActivationFunctionType.Tanh
IDENT = mybir.ActivationFunctionType.Identity
MULT = mybir.AluOpType.mult
ADD = mybir.AluOpType.add

D, S, O, T, B = 32, 8, 4, 256, 1024
N_CORES = 8
BC = B // N_CORES  # 128 batch per core
G = BC
WDT = np.float16

# ---------------------------------------------------------------------------
# walrus in this container rejects instructions carrying more than one sem
# wait ("Too many sync wait commands"). After Tile lowers everything, move
# surplus waits onto same-engine NOPs inserted just before each offender.
_MAXW = 1


def _split_waits(nc):
    for f in nc.m.functions:
        for blk in f.blocks:
            il = blk.instructions
            cur = list(il)
            out_list = []
            changed = False
            for ins in cur:
                si = ins.sync_info
                w = list(si.on_wait or []) if si is not None else []
                if len(w) > _MAXW:
                    changed = True
                    for i in range(0, len(w) - _MAXW, _MAXW):
                        bi = nc.engines[ins.engine].nop(nofuse=True)
                        nop_ins = bi.ins
                        for srch in (blk,) + tuple(f.blocks):
                            lst = srch.instructions
                            if lst and lst[-1] is nop_ins:
                                lst.pop()
                                break
                        nop_ins.sync_info = bass_rust.SyncInfo(
                            on_wait=w[i : i + _MAXW], on_update=[]
                        )
                        out_list.append(nop_ins)
                    si.on_wait = w[len(w) - _MAXW :]
                out_list.append(ins)
            if changed:
                il[:] = out_list


_orig_drain = tile.TileContext._drain_and_barrier


def _drain_then_split(self, tick_clock, wait_clock):
    _orig_drain(self, tick_clock, wait_clock)
    _split_waits(self.nc)


tile.TileContext._drain_and_barrier = _drain_then_split

# ---------------------------------------------------------------------------
WEIGHT_SPECS = [
    ("wa1", [96, 64], F16),  # cols: mult | attn
    ("wa2", [96, 32], F16),  # cols: W_all
    ("wb", [96, 96], F16),  # cols: multI | r | z
    ("wc", [96, 32], F16),  # rows: hW | hI | hA
    ("wd", [97, 64], F16),  # cols: A_all | multA; row 96 = a_b (ones row)
    ("wphi", [96, 4], F16),  # rows 64:96 = phi_w.T (matches A-slot base)
    ("wenc", [32, 128], F16),  # 4x block-diag enc_w.T
    ("biases", [128, 8], F32),
    ("id128", [128, 32], F16),  # 4x stacked identity
    ("id2", [64, 32], F16),  # [I; I]
]


def _pack_weights(inp):
    g = {k: np.ascontiguousarray(np.asarray(v, np.float32)) for k, v in inp.items()}

    wa = np.zeros((96, 96), np.float32)
    wa[32:64, 0:32] = g["wmg_w"][:, 0:32].T
    wa[64:96, 0:32] = g["wmg_w"][:, 32:64].T
    wa[0:32, 32:64] = g["att_w"][:, 0:32].T
    wa[32:64, 32:64] = g["att_w"][:, 32:64].T
    wa[0:32, 64:96] = g["wW"].T
    wa[32:64, 64:96] = g["wI"].T
    wa[64:96, 64:96] = g["wA"].T

    zb = np.concatenate([g["zW"].T, g["zI"].T, g["zA"].T], axis=0)
    rb = np.concatenate([g["rW"].T, g["rI"].T, g["rA"].T], axis=0)
    mib = np.zeros((96, 32), np.float32)
    mib[0:32] = g["img_w"][:, 0:32].T
    mib[64:96] = g["img_w"][:, 32:64].T
    wb = np.concatenate([mib, rb, zb], axis=1)  # multI | r | z

    wc = np.concatenate([g["hW"].T, g["hI"].T, g["hA"].T], axis=0)

    aall = np.concatenate(
        [g["aW"].T, g["aI"].T, g["aA"].T, g["a_b"][None, :]], axis=0
    )
    mab = np.zeros((97, 32), np.float32)
    mab[0:32] = g["amg_w"][:, 0:32].T
    mab[32:64] = g["amg_w"][:, 32:64].T
    wd = np.concatenate([aall, mab], axis=1)

    wphi = np.zeros((96, 4), np.float32)
    wphi[64:96] = g["phi_w"].T

    wenc = np.zeros((32, 128), np.float32)
    for k in range(4):
        wenc[k * 8 : (k + 1) * 8, k * 32 : (k + 1) * 32] = g["enc_w"].T

    biases = np.zeros((128, 8), np.float32)
    biases[0:32, 0] = g["wmg_b"]
    biases[32:64, 0] = g["att_b"]
    biases[0:32, 1] = g["img_b"]
    biases[32:64, 1] = g["r_b"]
    biases[64:96, 1] = g["z_b"]
    biases[0:32, 2] = g["h_b"]
    biases[32:64, 3] = g["amg_b"]
    biases[0:4, 6] = g["phi_b"]
    biases[:, 5] = np.tile(g["enc_b"], 4)

    id32 = np.eye(32, dtype=np.float32)
    w = dict(
        wa1=np.ascontiguousarray(wa[:, 0:64]),
        wa2=np.ascontiguousarray(wa[:, 64:96]),
        wb=wb, wc=wc, wd=wd, wphi=wphi, wenc=wenc, biases=biases,
        id128=np.tile(id32, (4, 1)),
        id2=np.concatenate([id32, id32], axis=0),
    )
    return {
        k: np.ascontiguousarray(v if k == "biases" else v.astype(WDT))
        for k, v in w.items()
    }


def _pack_obs_shard(obs_shard):
    """[T, BC, S] f32 -> [32, T/4*BC] fp16: row k*8+s, col c*BC+b holds
    obs[4c+k, b, s] (4 timesteps stacked on partitions)."""
    x = np.ascontiguousarray(obs_shard).reshape(T // 4, 4, BC, S)
    x = x.transpose(1, 3, 0, 2)
    return np.ascontiguousarray(x.reshape(32, (T // 4) * BC)).astype(WDT)


def _unpack_out(out_core):
    """[4, T*BC] -> [T, BC, O]."""
    return np.ascontiguousarray(out_core.reshape(O, T, BC).transpose(1, 2, 0))


def _build_nc():
    nc = bass.Bass()
    obs4 = nc.declare_dram_parameter("obs4", [32, (T // 4) * BC], F16, isOutput=False)
    wdram = {}
    for name, shape, dt in WEIGHT_SPECS:
        wdram[name] = nc.declare_dram_parameter(name, shape, dt, isOutput=False)
    out = nc.declare_dram_parameter("out", [4, T * BC], F32, isOutput=True)

    with tile.TileContext(nc) as tc:
        with (
            tc.tile_pool(name="singles", bufs=1) as singles,
            tc.tile_pool(name="psum", bufs=1, space="PSUM") as psum,
            tc.tile_pool(name="outp", bufs=3) as outp,
        ):
            wsb = {}
            for name, shape, dt in WEIGHT_SPECS:
                wsb[name] = singles.tile(shape, dt, name=f"w_{name}")
                nc.sync.dma_start(out=wsb[name], in_=wdram[name][:, :])
            obs_sb = singles.tile([32, (T // 4) * BC], F16)
            nc.sync.dma_start(out=obs_sb, in_=obs4[:, :])

            bia = wsb["biases"]

            # obs_enc_all = tanh(wenc.T @ obs4 + enc_b), all steps up front
            oenc = singles.tile([128, (T // 4) * BC], F16)
            NPRE = (T // 4) * BC // 512
            with tc.tile_pool(name="psum_pre", bufs=1, space="PSUM") as psum_pre:
                for i in range(NPRE):
                    ppre = psum_pre.tile([128, 512], F32)
                    nc.tensor.matmul(
                        ppre, wsb["wenc"], obs_sb[:, i * 512 : (i + 1) * 512],
                        start=True, stop=True,
                    )
                    nc.scalar.activation(
                        out=oenc[:, i * 512 : (i + 1) * 512], in_=ppre,
                        func=TANH, bias=bia[:, 5:6],
                    )

            # SB-SB elementwise inputs must share a start partition; outputs
            # are free. fp16 SBUF ops hit the DVE 2x mode.
            NB = 1
            g1 = [singles.tile([64, G], F16, name=f"g1_{g}") for g in range(NB)]
            prod = [singles.tile([64, G], F16, name=f"prod_{g}") for g in range(NB)]
            g2s = [singles.tile([96, G], F16, name=f"g2s_{g}") for g in range(NB)]
            g3t = [singles.tile([64, G], F16, name=f"g3t_{g}") for g in range(NB)]
            ht = [singles.tile([96, G], F16, name=f"ht_{g}") for g in range(NB)]
            omzt = [singles.tile([64, G], F16, name=f"omzt_{g}") for g in range(NB)]
            scr = [singles.tile([64, 2 * G], F16, name=f"scr_{g}") for g in range(NB)]
            icp = [singles.tile([64, G], F16, name=f"icp_{g}") for g in range(NB)]
            stkg = [singles.tile([128, G], F16, name=f"stk_{g}") for g in range(1)]
            nc.vector.memset(stkg[0], 0.0)
            nc.vector.memset(stkg[0][96:97, :], 1.0)  # ones row for bias folds

            # PSUM banks, packed so co-resident tensors are never PE-written
            # while another is engine-read concurrently.
            bankA = [psum.tile([128, 128], F32, name=f"bankA{g}") for g in range(1)]
            bankB = [psum.tile([96, 512], F32, name=f"bankB{g}") for g in range(1)]
            p4 = psum.tile([4, 512], F32)

            stk = stkg[0]
            s96 = stk[0:96, :]
            s97 = stk[0:97, :]

            def emit_phi(t):
                # phi matmul for step t, deferred into step t+1's sigma1
                # window so it never blocks the PE FIFO on the chain.
                k = t % 4
                nc.tensor.matmul(
                    p4[0:4, k * BC : (k + 1) * BC],
                    wsb["wphi"][64:96, :], stk[64:96, :],
                    start=True, stop=True,
                )

            def emit_evac(t):
                # evacuate on ScalarE (Identity + phi_b), emitted after
                # sigma1 so it lands in ACT's idle gap, off the DVE FIFO.
                if t < 0 or t % 4 != 3:
                    return
                ch = outp.tile([4, 512], F32)
                nc.scalar.activation(out=ch, in_=p4, func=IDENT, bias=bia[0:4, 6:7])
                nc.sync.dma_start(
                    out=out[0:4, (t // 4) * 512 : (t // 4 + 1) * 512], in_=ch
                )

            for t in range(T):
                c, k = t // 4, t % 4
                b = 0
                p0 = bankA[b][:, 0:G]  # [mult; attn; W_all; oe] rows
                p1 = bankB[b][:, 0:G]
                p2w = bankB[b][0:32, G : 2 * G]
                p2h = bankB[b][0:32, 2 * G : 3 * G]
                p3 = bankB[b][0:64, 3 * G : 4 * G]
                gg1 = g1[b]
                gprod = prod[b]
                gg2 = g2s[b]
                gg3 = g3t[b][32:64, :]
                gh = ht[b][0:32, :]  # matches multI base 0
                gomz = omzt[b][32:64, :]  # matches I base 32
                gv = scr[b][32:64, 0:G]
                gu = scr[b][32:64, G : 2 * G]
                ghm = ht[b][64:96, :]  # matches z base 64
                gic = icp[b][32:64, :]  # I snapshot, base 32
                oe = oenc[k * 32 : (k + 1) * 32, c * BC : (c + 1) * BC]
                # --- phase A: p0 = [mult_pre; attn_pre; W_all; oe] ---
                if t == 0:
                    nc.tensor.matmul(
                        p0[96:128, :], wsb["id128"][k * 32 : (k + 1) * 32, :], oe,
                        start=True, stop=True, tile_position=(k * 32, 96),
                    )
                nc.tensor.matmul(p0[0:64, :], wsb["wa1"], s96, start=True, stop=True)
                nc.tensor.matmul(p0[64:96, :], wsb["wa2"], s96, start=True, stop=True, tile_position=(0, 64))
                if t > 0:
                    emit_phi(t - 1)  # runs on PE during sigma1
                nc.scalar.activation(out=gg1, in_=p0[0:64, :], func=SIG, bias=bia[0:64, 0:1])
                if t > 0:
                    emit_evac(t - 1)
                nc.vector.tensor_tensor(out=gprod, in0=gg1, in1=p0[64:128, :], op=MULT)
                nc.vector.tensor_copy(out=gic, in_=stk[32:64, :])  # I snapshot
                nc.tensor.matmul(p2w, wsb["id2"], gprod, start=True, stop=True)
                nc.scalar.activation(out=stk[0:32, :], in_=p2w, func=TANH)  # W_new
                # --- phase B ---
                nc.tensor.matmul(p1, wsb["wb"], s96, start=True, stop=True)
                # [multI; r] half first: rI and mmC launch immediately
                nc.scalar.activation(out=gg2[0:64, :], in_=p1[0:64, :], func=SIG, bias=bia[0:64, 1:2])
                nc.vector.tensor_tensor(out=stk[32:64, :], in0=gg2[32:64, :], in1=stk[32:64, :], op=MULT)
                nc.tensor.matmul(p2h, wsb["wc"], s96, start=True, stop=True)
                # z half + omz + v hide inside the mmC/tanhH window
                nc.scalar.activation(out=gg2[64:96, :], in_=p1[64:96, :], func=SIG, bias=bia[64:96, 1:2])
                nc.vector.tensor_scalar(
                    out=gomz, in0=gg2[64:96, :], scalar1=-1.0, scalar2=1.0,
                    op0=MULT, op1=ADD,
                )
                nc.vector.tensor_tensor(out=gv, in0=gomz, in1=gic, op=MULT)
                nc.scalar.activation(out=gh, in_=p2h, func=TANH, bias=bia[0:32, 2:3])
                nc.vector.tensor_tensor(out=ghm, in0=gh, in1=gg2[0:32, :], op=MULT)
                nc.vector.tensor_tensor(out=gu, in0=ghm, in1=gg2[64:96, :], op=MULT)
                nc.vector.tensor_tensor(out=stk[32:64, :], in0=gu, in1=gv, op=ADD)  # I_new
                # --- phase C (a_b rides the ones row through wd) ---
                nc.tensor.matmul(p3, wsb["wd"], s97, start=True, stop=True)
                nc.scalar.activation(out=gg3, in_=p3[32:64, :], func=SIG, bias=bia[32:64, 3:4])
                nc.vector.tensor_tensor(out=p3[0:32, :], in0=p3[0:32, :], in1=gg3, op=MULT)
                if t + 1 < T:
                    # next step's obs_enc inject, off the critical chain
                    c2, k2 = (t + 1) // 4, (t + 1) % 4
                    oe2 = oenc[k2 * 32 : (k2 + 1) * 32, c2 * BC : (c2 + 1) * BC]
                    nc.tensor.matmul(
                        p0[96:128, :], wsb["id128"][k2 * 32 : (k2 + 1) * 32, :], oe2,
                        start=True, stop=True, tile_position=(k2 * 32, 96),
                    )
                nc.scalar.activation(out=stk[64:96, :], in_=p3[0:32, :], func=TANH)  # A_new
            emit_phi(T - 1)
            emit_evac(T - 1)
    return nc


_NC_CACHE = None


def kernel(**inputs):
    global _NC_CACHE
    from concourse.bass_utils import run_bass_kernel_spmd

    obs = np.ascontiguousarray(np.asarray(inputs["obs"], np.float32))
    w = _pack_weights({k: v for k, v in inputs.items() if k != "obs"})

    if _NC_CACHE is None:
        _NC_CACHE = _build_nc()
    nc = _NC_CACHE

    in_maps = []
    for i in range(N_CORES):
        m = dict(w)
        m["obs4"] = _pack_obs_shard(obs[:, i * BC : (i + 1) * BC, :])
        in_maps.append(m)

    res = run_bass_kernel_spmd(
        nc, in_maps, core_ids=list(range(N_CORES)), trace=False
    )
    outs = [_unpack_out(np.asarray(res.results[i]["out"])) for i in range(N_CORES)]
    return np.concatenate(outs, axis=1).astype(np.float32)  # [T, B, O]


# revision 6
# speedup vs baseline: 1.0284x; 1.0284x over previous
"""ANIMAZero recurrent cell on 8 TRN2 NeuronCores (Bass/Tile).

Data-parallel: batch 1024 is split into 8 shards of 128; each core runs
the full T=256 recurrence on its shard. Per step, the three D=32 states
[W; I; A] live stacked on SBUF partitions so each gate group is one
fp16 matmul; sigmoid/tanh run on ScalarE with fused per-partition
biases; elementwise gating runs on VectorE in fp16 2x mode. The phi
output projection accumulates 4 steps in a PSUM bank and is evacuated
on ScalarE into the per-step idle window.
"""

import os
import sys

sys.path.insert(0, "/opt/trn_rl_repo")
import numpy as np
import bass_rust
import concourse.bass as bass
import concourse.tile as tile
from concourse import mybir

F32 = mybir.dt.float32
F16 = mybir.dt.float16
SIG = mybir.ActivationFunctionType.Sigmoid
TANH = mybir.ActivationFunctionType.Tanh
IDENT = mybir.ActivationFunctionType.Identity
MULT = mybir.AluOpType.mult
ADD = mybir.AluOpType.add

D, S, O, T, B = 32, 8, 4, 256, 1024
N_CORES = 8
BC = B // N_CORES  # 128 batch per core
G = BC
WDT = np.float16

# ---------------------------------------------------------------------------
# walrus in this container rejects instructions carrying more than one sem
# wait ("Too many sync wait commands"). After Tile lowers everything, move
# surplus waits onto same-engine NOPs inserted just before each offender.
_MAXW = 1


def _split_waits(nc):
    for f in nc.m.functions:
        for blk in f.blocks:
            il = blk.instructions
            cur = list(il)
            out_list = []
            changed = False
            for ins in cur:
                si = ins.sync_info
                w = list(si.on_wait or []) if si is not None else []
                if len(w) > _MAXW:
                    changed = True
                    for i in range(0, len(w) - _MAXW, _MAXW):
                        bi = nc.engines[ins.engine].nop(nofuse=True)
                        nop_ins = bi.ins
                        for srch in (blk,) + tuple(f.blocks):
                            lst = srch.instructions
                            if lst and lst[-1] is nop_ins:
                                lst.pop()
                                break
                        nop_ins.sync_info = bass_rust.SyncInfo(
                            on_wait=w[i : i + _MAXW], on_update=[]
                        )
                        out_list.append(nop_ins)
                    si.on_wait = w[len(w) - _MAXW :]
                out_list.append(ins)
            if changed:
                il[:] = out_list


_orig_drain = tile.TileContext._drain_and_barrier


def _drain_then_split(self, tick_clock, wait_clock):
    _orig_drain(self, tick_clock, wait_clock)
    _split_waits(self.nc)


tile.TileContext._drain_and_barrier = _drain_then_split

# ---------------------------------------------------------------------------
WEIGHT_SPECS = [
    ("wa1", [96, 64], F16),  # cols: mult | attn
    ("wa2", [96, 32], F16),  # cols: W_all
    ("wb", [96, 96], F16),  # cols: z | r | multI
    ("wc", [96, 32], F16),  # rows: hW | hI | hA
    ("wd", [97, 64], F16),  # cols: A_all | multA; row 96 = a_b (ones row)
    ("wphi", [96, 4], F16),  # rows 64:96 = phi_w.T (matches A-slot base)
    ("wenc", [32, 128], F16),  # 4x block-diag enc_w.T
    ("biases", [128, 8], F32),
    ("id128", [128, 32], F16),  # 4x stacked identity
    ("id2", [64, 32], F16),  # [I; I]
]


def _pack_weights(inp):
    g = {k: np.ascontiguousarray(np.asarray(v, np.float32)) for k, v in inp.items()}

    wa = np.zeros((96, 96), np.float32)
    wa[32:64, 0:32] = g["wmg_w"][:, 0:32].T
    wa[64:96, 0:32] = g["wmg_w"][:, 32:64].T
    wa[0:32, 32:64] = g["att_w"][:, 0:32].T
    wa[32:64, 32:64] = g["att_w"][:, 32:64].T
    wa[0:32, 64:96] = g["wW"].T
    wa[32:64, 64:96] = g["wI"].T
    wa[64:96, 64:96] = g["wA"].T

    zb = np.concatenate([g["zW"].T, g["zI"].T, g["zA"].T], axis=0)
    rb = np.concatenate([g["rW"].T, g["rI"].T, g["rA"].T], axis=0)
    mib = np.zeros((96, 32), np.float32)
    mib[0:32] = g["img_w"][:, 0:32].T
    mib[64:96] = g["img_w"][:, 32:64].T
    wb = np.concatenate([zb, rb, mib], axis=1)

    wc = np.concatenate([g["hW"].T, g["hI"].T, g["hA"].T], axis=0)

    aall = np.concatenate(
        [g["aW"].T, g["aI"].T, g["aA"].T, g["a_b"][None, :]], axis=0
    )
    mab = np.zeros((97, 32), np.float32)
    mab[0:32] = g["amg_w"][:, 0:32].T
    mab[32:64] = g["amg_w"][:, 32:64].T
    wd = np.concatenate([aall, mab], axis=1)

    wphi = np.zeros((96, 4), np.float32)
    wphi[64:96] = g["phi_w"].T

    wenc = np.zeros((32, 128), np.float32)
    for k in range(4):
        wenc[k * 8 : (k + 1) * 8, k * 32 : (k + 1) * 32] = g["enc_w"].T

    biases = np.zeros((128, 8), np.float32)
    biases[0:32, 0] = g["wmg_b"]
    biases[32:64, 0] = g["att_b"]
    biases[0:32, 1] = g["z_b"]
    biases[32:64, 1] = g["r_b"]
    biases[64:96, 1] = g["img_b"]
    biases[0:32, 2] = g["h_b"]
    biases[32:64, 3] = g["amg_b"]
    biases[0:4, 6] = g["phi_b"]
    biases[:, 5] = np.tile(g["enc_b"], 4)

    id32 = np.eye(32, dtype=np.float32)
    w = dict(
        wa1=np.ascontiguousarray(wa[:, 0:64]),
        wa2=np.ascontiguousarray(wa[:, 64:96]),
        wb=wb, wc=wc, wd=wd, wphi=wphi, wenc=wenc, biases=biases,
        id128=np.tile(id32, (4, 1)),
        id2=np.concatenate([id32, id32], axis=0),
    )
    return {
        k: np.ascontiguousarray(v if k == "biases" else v.astype(WDT))
        for k, v in w.items()
    }


def _pack_obs_shard(obs_shard):
    """[T, BC, S] f32 -> [32, T/4*BC] fp16: row k*8+s, col c*BC+b holds
    obs[4c+k, b, s] (4 timesteps stacked on partitions)."""
    x = np.ascontiguousarray(obs_shard).reshape(T // 4, 4, BC, S)
    x = x.transpose(1, 3, 0, 2)
    return np.ascontiguousarray(x.reshape(32, (T // 4) * BC)).astype(WDT)


def _unpack_out(out_core):
    """[4, T*BC] -> [T, BC, O]."""
    return np.ascontiguousarray(out_core.reshape(O, T, BC).transpose(1, 2, 0))


def _build_nc():
    nc = bass.Bass()
    obs4 = nc.declare_dram_parameter("obs4", [32, (T // 4) * BC], F16, isOutput=False)
    wdram = {}
    for name, shape, dt in WEIGHT_SPECS:
        wdram[name] = nc.declare_dram_parameter(name, shape, dt, isOutput=False)
    out = nc.declare_dram_parameter("out", [4, T * BC], F32, isOutput=True)

    with tile.TileContext(nc) as tc:
        with (
            tc.tile_pool(name="singles", bufs=1) as singles,
            tc.tile_pool(name="psum", bufs=1, space="PSUM") as psum,
            tc.tile_pool(name="outp", bufs=3) as outp,
        ):
            wsb = {}
            for name, shape, dt in WEIGHT_SPECS:
                wsb[name] = singles.tile(shape, dt, name=f"w_{name}")
                nc.sync.dma_start(out=wsb[name], in_=wdram[name][:, :])
            obs_sb = singles.tile([32, (T // 4) * BC], F16)
            nc.sync.dma_start(out=obs_sb, in_=obs4[:, :])

            bia = wsb["biases"]

            # obs_enc_all = tanh(wenc.T @ obs4 + enc_b), all steps up front
            oenc = singles.tile([128, (T // 4) * BC], F16)
            NPRE = (T // 4) * BC // 512
            with tc.tile_pool(name="psum_pre", bufs=1, space="PSUM") as psum_pre:
                for i in range(NPRE):
                    ppre = psum_pre.tile([128, 512], F32)
                    nc.tensor.matmul(
                        ppre, wsb["wenc"], obs_sb[:, i * 512 : (i + 1) * 512],
                        start=True, stop=True,
                    )
                    nc.scalar.activation(
                        out=oenc[:, i * 512 : (i + 1) * 512], in_=ppre,
                        func=TANH, bias=bia[:, 5:6],
                    )

            # SB-SB elementwise inputs must share a start partition; outputs
            # are free. fp16 SBUF ops hit the DVE 2x mode.
            NB = 1
            g1 = [singles.tile([64, G], F16, name=f"g1_{g}") for g in range(NB)]
            prod = [singles.tile([64, G], F16, name=f"prod_{g}") for g in range(NB)]
            g2s = [singles.tile([96, G], F16, name=f"g2s_{g}") for g in range(NB)]
            g3t = [singles.tile([64, G], F16, name=f"g3t_{g}") for g in range(NB)]
            ht = [singles.tile([96, G], F16, name=f"ht_{g}") for g in range(NB)]
            omzt = [singles.tile([64, G], F16, name=f"omzt_{g}") for g in range(NB)]
            scr = [singles.tile([64, 2 * G], F16, name=f"scr_{g}") for g in range(NB)]
            hmt = [singles.tile([32, G], F16, name=f"hmt_{g}") for g in range(NB)]
            stkg = [singles.tile([128, G], F16, name=f"stk_{g}") for g in range(1)]
            nc.vector.memset(stkg[0], 0.0)
            nc.vector.memset(stkg[0][96:97, :], 1.0)  # ones row for bias folds

            # PSUM banks, packed so co-resident tensors are never PE-written
            # while another is engine-read concurrently.
            bankA = [psum.tile([128, 128], F32, name=f"bankA{g}") for g in range(1)]
            bankB = [psum.tile([96, 512], F32, name=f"bankB{g}") for g in range(1)]
            p4 = psum.tile([4, 512], F32)

            stk = stkg[0]
            s96 = stk[0:96, :]
            s97 = stk[0:97, :]

            def emit_phi(t):
                # phi matmul for step t, deferred into step t+1's sigma1
                # window so it never blocks the PE FIFO on the chain.
                k = t % 4
                nc.tensor.matmul(
                    p4[0:4, k * BC : (k + 1) * BC],
                    wsb["wphi"][64:96, :], stk[64:96, :],
                    start=True, stop=True,
                )

            def emit_evac(t):
                # evacuate on ScalarE (Identity + phi_b), emitted after
                # sigma1 so it lands in ACT's idle gap, off the DVE FIFO.
                if t < 0 or t % 4 != 3:
                    return
                ch = outp.tile([4, 512], F32)
                nc.scalar.activation(out=ch, in_=p4, func=IDENT, bias=bia[0:4, 6:7])
                nc.sync.dma_start(
                    out=out[0:4, (t // 4) * 512 : (t // 4 + 1) * 512], in_=ch
                )

            for t in range(T):
                c, k = t // 4, t % 4
                b = 0
                p0 = bankA[b][:, 0:G]  # [mult; attn; W_all; oe] rows
                p1 = bankB[b][:, 0:G]
                p2w = bankB[b][0:32, G : 2 * G]
                p2h = bankB[b][0:32, 2 * G : 3 * G]
                p3 = bankB[b][0:64, 3 * G : 4 * G]
                gg1 = g1[b]
                gprod = prod[b]
                gg2 = g2s[b]
                gg3 = g3t[b][32:64, :]
                gh = ht[b][64:96, :]  # matches multI base 64
                gomz = omzt[b][32:64, :]  # matches I base 32
                gv = scr[b][32:64, 0:G]
                gu = scr[b][32:64, G : 2 * G]
                ghm = hmt[b]
                oe = oenc[k * 32 : (k + 1) * 32, c * BC : (c + 1) * BC]
                # --- phase A: p0 = [mult_pre; attn_pre; W_all; oe] ---
                if t == 0:
                    nc.tensor.matmul(
                        p0[96:128, :], wsb["id128"][k * 32 : (k + 1) * 32, :], oe,
                        start=True, stop=True, tile_position=(k * 32, 96),
                    )
                nc.tensor.matmul(p0[0:64, :], wsb["wa1"], s96, start=True, stop=True)
                nc.tensor.matmul(p0[64:96, :], wsb["wa2"], s96, start=True, stop=True, tile_position=(0, 64))
                if t > 0:
                    emit_phi(t - 1)  # runs on PE during sigma1
                nc.scalar.activation(out=gg1, in_=p0[0:64, :], func=SIG, bias=bia[0:64, 0:1])
                if t > 0:
                    emit_evac(t - 1)
                nc.vector.tensor_tensor(out=gprod, in0=gg1, in1=p0[64:128, :], op=MULT)
                nc.tensor.matmul(p2w, wsb["id2"], gprod, start=True, stop=True)
                nc.scalar.activation(out=stk[0:32, :], in_=p2w, func=TANH)  # W_new
                # --- phase B ---
                nc.tensor.matmul(p1, wsb["wb"], s96, start=True, stop=True)
                nc.scalar.activation(out=gg2, in_=p1, func=SIG, bias=bia[0:96, 1:2])
                nc.vector.tensor_scalar(
                    out=gomz, in0=gg2[0:32, :], scalar1=-1.0, scalar2=1.0,
                    op0=MULT, op1=ADD,
                )
                nc.vector.tensor_tensor(out=gv, in0=gomz, in1=stk[32:64, :], op=MULT)
                # I is dead after v: r*I overwrites the I-slot in place so
                # the h matmul is one contiguous K=96 contraction.
                nc.vector.tensor_tensor(out=stk[32:64, :], in0=gg2[32:64, :], in1=stk[32:64, :], op=MULT)
                nc.tensor.matmul(p2h, wsb["wc"], s96, start=True, stop=True)
                nc.scalar.activation(out=gh, in_=p2h, func=TANH, bias=bia[0:32, 2:3])
                nc.vector.tensor_tensor(out=ghm, in0=gh, in1=gg2[64:96, :], op=MULT)
                nc.vector.tensor_tensor(out=gu, in0=ghm, in1=gg2[0:32, :], op=MULT)
                nc.vector.tensor_tensor(out=stk[32:64, :], in0=gu, in1=gv, op=ADD)  # I_new
                # --- phase C (a_b rides the ones row through wd) ---
                nc.tensor.matmul(p3, wsb["wd"], s97, start=True, stop=True)
                nc.scalar.activation(out=gg3, in_=p3[32:64, :], func=SIG, bias=bia[32:64, 3:4])
                nc.vector.tensor_tensor(out=p3[0:32, :], in0=p3[0:32, :], in1=gg3, op=MULT)
                if t + 1 < T:
                    # next step's obs_enc inject, off the critical chain
                    c2, k2 = (t + 1) // 4, (t + 1) % 4
                    oe2 = oenc[k2 * 32 : (k2 + 1) * 32, c2 * BC : (c2 + 1) * BC]
                    nc.tensor.matmul(
                        p0[96:128, :], wsb["id128"][k2 * 32 : (k2 + 1) * 32, :], oe2,
                        start=True, stop=True, tile_position=(k2 * 32, 96),
                    )
                nc.scalar.activation(out=stk[64:96, :], in_=p3[0:32, :], func=TANH)  # A_new
            emit_phi(T - 1)
            emit_evac(T - 1)
    return nc


_NC_CACHE = None


def kernel(**inputs):
    global _NC_CACHE
    from concourse.bass_utils import run_bass_kernel_spmd

    obs = np.ascontiguousarray(np.asarray(inputs["obs"], np.float32))
    w = _pack_weights({k: v for k, v in inputs.items() if k != "obs"})

    if _NC_CACHE is None:
        _NC_CACHE = _build_nc()
    nc = _NC_CACHE

    in_maps = []
    for i in range(N_CORES):
        m = dict(w)
        m["obs4"] = _pack_obs_shard(obs[:, i * BC : (i + 1) * BC, :])
        in_maps.append(m)

    res = run_bass_kernel_spmd(
        nc, in_maps, core_ids=list(range(N_CORES)), trace=False
    )
    outs = [_unpack_out(np.asarray(res.results[i]["out"])) for i in range(N_CORES)]
    return np.concatenate(outs, axis=1).astype(np.float32)  # [T, B, O]


# revision 7
# speedup vs baseline: 1.0341x; 1.0055x over previous
"""ANIMAZero recurrent cell on 8 TRN2 NeuronCores (Bass/Tile).

Data-parallel: batch 1024 is split into 8 shards of 128; each core runs
the full T=256 recurrence on its shard. Per step, the three D=32 states
[W; I; A] live stacked on SBUF partitions so each gate group is one
fp16 matmul; sigmoid/tanh run on ScalarE with fused per-partition
biases; elementwise gating runs on VectorE in fp16 2x mode. The phi
output projection accumulates 4 steps in a PSUM bank and is evacuated
on ScalarE into the per-step idle window.
"""

import os
import sys

sys.path.insert(0, "/opt/trn_rl_repo")
import numpy as np
import bass_rust
import concourse.bass as bass
import concourse.tile as tile
from concourse import mybir

F32 = mybir.dt.float32
F16 = mybir.dt.float16
SIG = mybir.ActivationFunctionType.Sigmoid
TANH = mybir.ActivationFunctionType.Tanh
IDENT = mybir.ActivationFunctionType.Identity
MULT = mybir.AluOpType.mult
ADD = mybir.AluOpType.add

D, S, O, T, B = 32, 8, 4, 256, 1024
N_CORES = 8
BC = B // N_CORES  # 128 batch per core
G = BC
WDT = np.float16

# ---------------------------------------------------------------------------
# walrus in this container rejects instructions carrying more than one sem
# wait ("Too many sync wait commands"). After Tile lowers everything, move
# surplus waits onto same-engine NOPs inserted just before each offender.
_MAXW = 1


def _split_waits(nc):
    for f in nc.m.functions:
        for blk in f.blocks:
            il = blk.instructions
            cur = list(il)
            out_list = []
            changed = False
            for ins in cur:
                si = ins.sync_info
                w = list(si.on_wait or []) if si is not None else []
                if len(w) > _MAXW:
                    changed = True
                    for i in range(0, len(w) - _MAXW, _MAXW):
                        bi = nc.engines[ins.engine].nop(nofuse=True)
                        nop_ins = bi.ins
                        for srch in (blk,) + tuple(f.blocks):
                            lst = srch.instructions
                            if lst and lst[-1] is nop_ins:
                                lst.pop()
                                break
                        nop_ins.sync_info = bass_rust.SyncInfo(
                            on_wait=w[i : i + _MAXW], on_update=[]
                        )
                        out_list.append(nop_ins)
                    si.on_wait = w[len(w) - _MAXW :]
                out_list.append(ins)
            if changed:
                il[:] = out_list


_orig_drain = tile.TileContext._drain_and_barrier


def _drain_then_split(self, tick_clock, wait_clock):
    _orig_drain(self, tick_clock, wait_clock)
    _split_waits(self.nc)


tile.TileContext._drain_and_barrier = _drain_then_split

# ---------------------------------------------------------------------------
WEIGHT_SPECS = [
    ("wa1", [96, 64], F16),  # cols: mult | attn
    ("wa2", [96, 32], F16),  # cols: W_all
    ("wb", [96, 96], F16),  # cols: z | r | multI
    ("wc", [96, 32], F16),  # rows: hW | hI | hA
    ("wd", [97, 64], F16),  # cols: A_all | multA; row 96 = a_b (ones row)
    ("wphi", [96, 4], F16),  # rows 64:96 = phi_w.T (matches A-slot base)
    ("wenc", [32, 128], F16),  # 4x block-diag enc_w.T
    ("biases", [128, 8], F32),
    ("id128", [128, 32], F16),  # 4x stacked identity
    ("id2", [64, 32], F16),  # [I; I]
]


def _pack_weights(inp):
    g = {k: np.ascontiguousarray(np.asarray(v, np.float32)) for k, v in inp.items()}

    wa = np.zeros((96, 96), np.float32)
    wa[32:64, 0:32] = g["wmg_w"][:, 0:32].T
    wa[64:96, 0:32] = g["wmg_w"][:, 32:64].T
    wa[0:32, 32:64] = g["att_w"][:, 0:32].T
    wa[32:64, 32:64] = g["att_w"][:, 32:64].T
    wa[0:32, 64:96] = g["wW"].T
    wa[32:64, 64:96] = g["wI"].T
    wa[64:96, 64:96] = g["wA"].T

    zb = np.concatenate([g["zW"].T, g["zI"].T, g["zA"].T], axis=0)
    rb = np.concatenate([g["rW"].T, g["rI"].T, g["rA"].T], axis=0)
    mib = np.zeros((96, 32), np.float32)
    mib[0:32] = g["img_w"][:, 0:32].T
    mib[64:96] = g["img_w"][:, 32:64].T
    wb = np.concatenate([zb, rb, mib], axis=1)

    wc = np.concatenate([g["hW"].T, g["hI"].T, g["hA"].T], axis=0)

    aall = np.concatenate(
        [g["aW"].T, g["aI"].T, g["aA"].T, g["a_b"][None, :]], axis=0
    )
    mab = np.zeros((97, 32), np.float32)
    mab[0:32] = g["amg_w"][:, 0:32].T
    mab[32:64] = g["amg_w"][:, 32:64].T
    wd = np.concatenate([aall, mab], axis=1)

    wphi = np.zeros((96, 4), np.float32)
    wphi[64:96] = g["phi_w"].T

    wenc = np.zeros((32, 128), np.float32)
    for k in range(4):
        wenc[k * 8 : (k + 1) * 8, k * 32 : (k + 1) * 32] = g["enc_w"].T

    biases = np.zeros((128, 8), np.float32)
    biases[0:32, 0] = g["wmg_b"]
    biases[32:64, 0] = g["att_b"]
    biases[0:32, 1] = g["z_b"]
    biases[32:64, 1] = g["r_b"]
    biases[64:96, 1] = g["img_b"]
    biases[0:32, 2] = g["h_b"]
    biases[32:64, 3] = g["amg_b"]
    biases[0:4, 6] = g["phi_b"]
    biases[:, 5] = np.tile(g["enc_b"], 4)

    id32 = np.eye(32, dtype=np.float32)
    w = dict(
        wa1=np.ascontiguousarray(wa[:, 0:64]),
        wa2=np.ascontiguousarray(wa[:, 64:96]),
        wb=wb, wc=wc, wd=wd, wphi=wphi, wenc=wenc, biases=biases,
        id128=np.tile(id32, (4, 1)),
        id2=np.concatenate([id32, id32], axis=0),
    )
    return {
        k: np.ascontiguousarray(v if k == "biases" else v.astype(WDT))
        for k, v in w.items()
    }


def _pack_obs_shard(obs_shard):
    """[T, BC, S] f32 -> [32, T/4*BC] fp16: row k*8+s, col c*BC+b holds
    obs[4c+k, b, s] (4 timesteps stacked on partitions)."""
    x = np.ascontiguousarray(obs_shard).reshape(T // 4, 4, BC, S)
    x = x.transpose(1, 3, 0, 2)
    return np.ascontiguousarray(x.reshape(32, (T // 4) * BC)).astype(WDT)


def _unpack_out(out_core):
    """[4, T*BC] -> [T, BC, O]."""
    return np.ascontiguousarray(out_core.reshape(O, T, BC).transpose(1, 2, 0))


def _build_nc():
    nc = bass.Bass()
    obs4 = nc.declare_dram_parameter("obs4", [32, (T // 4) * BC], F16, isOutput=False)
    wdram = {}
    for name, shape, dt in WEIGHT_SPECS:
        wdram[name] = nc.declare_dram_parameter(name, shape, dt, isOutput=False)
    out = nc.declare_dram_parameter("out", [4, T * BC], F32, isOutput=True)

    with tile.TileContext(nc) as tc:
        with (
            tc.tile_pool(name="singles", bufs=1) as singles,
            tc.tile_pool(name="psum", bufs=1, space="PSUM") as psum,
            tc.tile_pool(name="outp", bufs=3) as outp,
        ):
            wsb = {}
            for name, shape, dt in WEIGHT_SPECS:
                wsb[name] = singles.tile(shape, dt, name=f"w_{name}")
                nc.sync.dma_start(out=wsb[name], in_=wdram[name][:, :])
            obs_sb = singles.tile([32, (T // 4) * BC], F16)
            nc.sync.dma_start(out=obs_sb, in_=obs4[:, :])

            bia = wsb["biases"]

            # obs_enc_all = tanh(wenc.T @ obs4 + enc_b), all steps up front
            oenc = singles.tile([128, (T // 4) * BC], F16)
            NPRE = (T // 4) * BC // 512
            with tc.tile_pool(name="psum_pre", bufs=1, space="PSUM") as psum_pre:
                for i in range(NPRE):
                    ppre = psum_pre.tile([128, 512], F32)
                    nc.tensor.matmul(
                        ppre, wsb["wenc"], obs_sb[:, i * 512 : (i + 1) * 512],
                        start=True, stop=True,
                    )
                    nc.scalar.activation(
                        out=oenc[:, i * 512 : (i + 1) * 512], in_=ppre,
                        func=TANH, bias=bia[:, 5:6],
                    )

            # SB-SB elementwise inputs must share a start partition; outputs
            # are free. fp16 SBUF ops hit the DVE 2x mode.
            NB = 1
            g1 = [singles.tile([64, G], F16, name=f"g1_{g}") for g in range(NB)]
            prod = [singles.tile([64, G], F16, name=f"prod_{g}") for g in range(NB)]
            g2s = [singles.tile([96, G], F16, name=f"g2s_{g}") for g in range(NB)]
            g3t = [singles.tile([64, G], F16, name=f"g3t_{g}") for g in range(NB)]
            ht = [singles.tile([96, G], F16, name=f"ht_{g}") for g in range(NB)]
            icp = [singles.tile([64, G], F16, name=f"icp_{g}") for g in range(NB)]
            scr = [singles.tile([64, 2 * G], F16, name=f"scr_{g}") for g in range(NB)]
            hmt = [singles.tile([32, G], F16, name=f"hmt_{g}") for g in range(NB)]
            stkg = [singles.tile([128, G], F16, name=f"stk_{g}") for g in range(1)]
            nc.vector.memset(stkg[0], 0.0)
            nc.vector.memset(stkg[0][96:97, :], 1.0)  # ones row for bias folds

            # PSUM banks, packed so co-resident tensors are never PE-written
            # while another is engine-read concurrently.
            bankA = [psum.tile([128, 128], F32, name=f"bankA{g}") for g in range(1)]
            bankB = [psum.tile([96, 512], F32, name=f"bankB{g}") for g in range(1)]
            p4 = psum.tile([4, 512], F32)

            stk = stkg[0]
            s96 = stk[0:96, :]
            s97 = stk[0:97, :]

            def emit_phi(t):
                # phi matmul for step t, deferred into step t+1's sigma1
                # window so it never blocks the PE FIFO on the chain.
                k = t % 4
                nc.tensor.matmul(
                    p4[0:4, k * BC : (k + 1) * BC],
                    wsb["wphi"][64:96, :], stk[64:96, :],
                    start=True, stop=True,
                )

            def emit_evac(t):
                # evacuate on ScalarE (Identity + phi_b), emitted after
                # sigma1 so it lands in ACT's idle gap, off the DVE FIFO.
                if t < 0 or t % 4 != 3:
                    return
                ch = outp.tile([4, 512], F32)
                nc.scalar.activation(out=ch, in_=p4, func=IDENT, bias=bia[0:4, 6:7])
                nc.sync.dma_start(
                    out=out[0:4, (t // 4) * 512 : (t // 4 + 1) * 512], in_=ch
                )

            for t in range(T):
                c, k = t // 4, t % 4
                b = 0
                p0 = bankA[b][:, 0:G]  # [mult; attn; W_all; oe] rows
                p1 = bankB[b][:, 0:G]
                p2w = bankB[b][0:32, G : 2 * G]
                p2h = bankB[b][0:32, 2 * G : 3 * G]
                p3 = bankB[b][0:64, 3 * G : 4 * G]
                gg1 = g1[b]
                gprod = prod[b]
                gg2 = g2s[b]
                gg3 = g3t[b][32:64, :]
                gh = ht[b][64:96, :]  # matches multI base 64
                gic = icp[b][32:64, :]  # I snapshot, base 32
                ghm = scr[b][32:64, 0:G]  # base 32, pairs the I snapshot
                gzd = scr[b][32:64, G : 2 * G]
                gd = hmt[b]  # base 0, pairs z
                oe = oenc[k * 32 : (k + 1) * 32, c * BC : (c + 1) * BC]
                # --- phase A: p0 = [mult_pre; attn_pre; W_all; oe] ---
                if t == 0:
                    nc.tensor.matmul(
                        p0[96:128, :], wsb["id128"][k * 32 : (k + 1) * 32, :], oe,
                        start=True, stop=True, tile_position=(k * 32, 96),
                    )
                nc.tensor.matmul(p0[0:64, :], wsb["wa1"], s96, start=True, stop=True)
                nc.tensor.matmul(p0[64:96, :], wsb["wa2"], s96, start=True, stop=True, tile_position=(0, 64))
                if t > 0:
                    emit_phi(t - 1)  # runs on PE during sigma1
                nc.scalar.activation(out=gg1, in_=p0[0:64, :], func=SIG, bias=bia[0:64, 0:1])
                if t > 0:
                    emit_evac(t - 1)
                nc.vector.tensor_tensor(out=gprod, in0=gg1, in1=p0[64:128, :], op=MULT)
                nc.vector.tensor_copy(out=gic, in_=stk[32:64, :])  # I snapshot
                nc.tensor.matmul(p2w, wsb["id2"], gprod, start=True, stop=True)
                nc.scalar.activation(out=stk[0:32, :], in_=p2w, func=TANH)  # W_new
                # --- phase B ---
                nc.tensor.matmul(p1, wsb["wb"], s96, start=True, stop=True)
                nc.scalar.activation(out=gg2, in_=p1, func=SIG, bias=bia[0:96, 1:2])
                # r*I overwrites the I-slot in place (snapshot taken above)
                # so the h matmul is one contiguous K=96 contraction.
                nc.vector.tensor_tensor(out=stk[32:64, :], in0=gg2[32:64, :], in1=stk[32:64, :], op=MULT)
                nc.tensor.matmul(p2h, wsb["wc"], s96, start=True, stop=True)
                nc.scalar.activation(out=gh, in_=p2h, func=TANH, bias=bia[0:32, 2:3])
                # I_new = I + z*(h*multI - I), via the base-32 snapshot
                nc.vector.tensor_tensor(out=ghm, in0=gh, in1=gg2[64:96, :], op=MULT)
                nc.vector.tensor_tensor(out=gd, in0=ghm, in1=gic, op=mybir.AluOpType.subtract)
                nc.vector.tensor_tensor(out=gzd, in0=gg2[0:32, :], in1=gd, op=MULT)
                nc.vector.tensor_tensor(out=stk[32:64, :], in0=gzd, in1=gic, op=ADD)  # I_new
                # --- phase C (a_b rides the ones row through wd) ---
                nc.tensor.matmul(p3, wsb["wd"], s97, start=True, stop=True)
                nc.scalar.activation(out=gg3, in_=p3[32:64, :], func=SIG, bias=bia[32:64, 3:4])
                nc.vector.tensor_tensor(out=p3[0:32, :], in0=p3[0:32, :], in1=gg3, op=MULT)
                if t + 1 < T:
                    # next step's obs_enc inject, off the critical chain
                    c2, k2 = (t + 1) // 4, (t + 1) % 4
                    oe2 = oenc[k2 * 32 : (k2 + 1) * 32, c2 * BC : (c2 + 1) * BC]
                    nc.tensor.matmul(
                        p0[96:128, :], wsb["id128"][k2 * 32 : (k2 + 1) * 32, :], oe2,
                        start=True, stop=True, tile_position=(k2 * 32, 96),
                    )
                nc.scalar.activation(out=stk[64:96, :], in_=p3[0:32, :], func=TANH)  # A_new
            emit_phi(T - 1)
            emit_evac(T - 1)
    return nc


_NC_CACHE = None


def kernel(**inputs):
    global _NC_CACHE
    from concourse.bass_utils import run_bass_kernel_spmd

    obs = np.ascontiguousarray(np.asarray(inputs["obs"], np.float32))
    w = _pack_weights({k: v for k, v in inputs.items() if k != "obs"})

    if _NC_CACHE is None:
        _NC_CACHE = _build_nc()
    nc = _NC_CACHE

    in_maps = []
    for i in range(N_CORES):
        m = dict(w)
        m["obs4"] = _pack_obs_shard(obs[:, i * BC : (i + 1) * BC, :])
        in_maps.append(m)

    res = run_bass_kernel_spmd(
        nc, in_maps, core_ids=list(range(N_CORES)), trace=False
    )
    outs = [_unpack_out(np.asarray(res.results[i]["out"])) for i in range(N_CORES)]
    return np.concatenate(outs, axis=1).astype(np.float32)  # [T, B, O]


# revision 8
# speedup vs baseline: 1.0760x; 1.0405x over previous
"""ANIMAZero recurrent cell on 8 TRN2 NeuronCores (Bass/Tile).

Data-parallel: batch 1024 is split into 8 shards of 128; each core runs
the full T=256 recurrence on its shard. Per step, the three D=32 states
[W; I; A] live stacked on SBUF partitions so each gate group is one
fp16 matmul; sigmoid/tanh run on ScalarE with fused per-partition
biases; elementwise gating runs on VectorE in fp16 2x mode. The phi
output projection accumulates 4 steps in a PSUM bank and is evacuated
on ScalarE into the per-step idle window.
"""

import os
import sys

sys.path.insert(0, "/opt/trn_rl_repo")
import numpy as np
import bass_rust
import concourse.bass as bass
import concourse.tile as tile
from concourse import mybir

F32 = mybir.dt.float32
F16 = mybir.dt.float16
SIG = mybir.ActivationFunctionType.Sigmoid
TANH = mybir.ActivationFunctionType.Tanh
IDENT = mybir.ActivationFunctionType.Identity
MULT = mybir.AluOpType.mult
ADD = mybir.AluOpType.add

D, S, O, T, B = 32, 8, 4, 256, 1024
N_CORES = 8
BC = B // N_CORES  # 128 batch per core
G = BC
WDT = np.float16

# ---------------------------------------------------------------------------
# walrus in this container rejects instructions carrying more than one sem
# wait ("Too many sync wait commands"). After Tile lowers everything, move
# surplus waits onto same-engine NOPs inserted just before each offender.
_MAXW = 1


def _split_waits(nc):
    for f in nc.m.functions:
        for blk in f.blocks:
            il = blk.instructions
            cur = list(il)
            out_list = []
            changed = False
            for ins in cur:
                si = ins.sync_info
                w = list(si.on_wait or []) if si is not None else []
                if len(w) > _MAXW:
                    changed = True
                    for i in range(0, len(w) - _MAXW, _MAXW):
                        bi = nc.engines[ins.engine].nop(nofuse=True)
                        nop_ins = bi.ins
                        for srch in (blk,) + tuple(f.blocks):
                            lst = srch.instructions
                            if lst and lst[-1] is nop_ins:
                                lst.pop()
                                break
                        nop_ins.sync_info = bass_rust.SyncInfo(
                            on_wait=w[i : i + _MAXW], on_update=[]
                        )
                        out_list.append(nop_ins)
                    si.on_wait = w[len(w) - _MAXW :]
                out_list.append(ins)
            if changed:
                il[:] = out_list


_orig_drain = tile.TileContext._drain_and_barrier


def _drain_then_split(self, tick_clock, wait_clock):
    _orig_drain(self, tick_clock, wait_clock)
    _split_waits(self.nc)


tile.TileContext._drain_and_barrier = _drain_then_split

# ---------------------------------------------------------------------------
WEIGHT_SPECS = [
    ("wa1", [96, 64], F16),  # cols: mult | attn
    ("wa2", [96, 32], F16),  # cols: W_all
    ("wb", [96, 96], F16),  # cols: z | r | multI
    ("wc", [96, 32], F16),  # rows: hW | hI | hA
    ("wd", [97, 64], F16),  # cols: A_all | multA; row 96 = a_b (ones row)
    ("wphi", [96, 4], F16),  # rows 64:96 = phi_w.T (matches A-slot base)
    ("wenc", [32, 128], F16),  # 4x block-diag enc_w.T
    ("biases", [128, 8], F32),
    ("id128", [128, 32], F16),  # 4x stacked identity
    ("id2", [64, 32], F16),  # [I; I]
]


def _pack_weights(inp):
    g = {k: np.ascontiguousarray(np.asarray(v, np.float32)) for k, v in inp.items()}

    wa = np.zeros((96, 96), np.float32)
    wa[32:64, 0:32] = g["wmg_w"][:, 0:32].T
    wa[64:96, 0:32] = g["wmg_w"][:, 32:64].T
    wa[0:32, 32:64] = g["att_w"][:, 0:32].T
    wa[32:64, 32:64] = g["att_w"][:, 32:64].T
    wa[0:32, 64:96] = g["wW"].T
    wa[32:64, 64:96] = g["wI"].T
    wa[64:96, 64:96] = g["wA"].T

    zb = np.concatenate([g["zW"].T, g["zI"].T, g["zA"].T], axis=0)
    rb = np.concatenate([g["rW"].T, g["rI"].T, g["rA"].T], axis=0)
    mib = np.zeros((96, 32), np.float32)
    mib[0:32] = g["img_w"][:, 0:32].T
    mib[64:96] = g["img_w"][:, 32:64].T
    wb = np.concatenate([zb, rb, mib], axis=1)

    wc = np.concatenate([g["hW"].T, g["hI"].T, g["hA"].T], axis=0)

    aall = np.concatenate(
        [g["aW"].T, g["aI"].T, g["aA"].T, g["a_b"][None, :]], axis=0
    )
    mab = np.zeros((97, 32), np.float32)
    mab[0:32] = g["amg_w"][:, 0:32].T
    mab[32:64] = g["amg_w"][:, 32:64].T
    wd = np.concatenate([aall, mab], axis=1)

    wphi = np.zeros((96, 4), np.float32)
    wphi[64:96] = g["phi_w"].T

    wenc = np.zeros((32, 128), np.float32)
    for k in range(4):
        wenc[k * 8 : (k + 1) * 8, k * 32 : (k + 1) * 32] = g["enc_w"].T

    biases = np.zeros((128, 8), np.float32)
    biases[0:32, 0] = g["wmg_b"]
    biases[32:64, 0] = g["att_b"]
    biases[0:32, 1] = g["z_b"]
    biases[32:64, 1] = g["r_b"]
    biases[64:96, 1] = g["img_b"]
    biases[0:32, 2] = g["h_b"]
    biases[32:64, 3] = g["amg_b"]
    biases[0:4, 6] = g["phi_b"]
    biases[:, 5] = np.tile(g["enc_b"], 4)

    id32 = np.eye(32, dtype=np.float32)
    w = dict(
        wa1=np.ascontiguousarray(wa[:, 0:64]),
        wa2=np.ascontiguousarray(wa[:, 64:96]),
        wb=wb, wc=wc, wd=wd, wphi=wphi, wenc=wenc, biases=biases,
        id128=np.tile(id32, (4, 1)),
        id2=np.concatenate([id32, id32], axis=0),
    )
    return {
        k: np.ascontiguousarray(v if k == "biases" else v.astype(WDT))
        for k, v in w.items()
    }


def _pack_obs_shard(obs_shard):
    """[T, BC, S] f32 -> [32, T/4*BC] fp16: row k*8+s, col c*BC+b holds
    obs[4c+k, b, s] (4 timesteps stacked on partitions)."""
    x = np.ascontiguousarray(obs_shard).reshape(T // 4, 4, BC, S)
    x = x.transpose(1, 3, 0, 2)
    return np.ascontiguousarray(x.reshape(32, (T // 4) * BC)).astype(WDT)


def _unpack_out(out_core):
    """[4, T*BC] -> [T, BC, O]."""
    return np.ascontiguousarray(out_core.reshape(O, T, BC).transpose(1, 2, 0))


def _build_nc():
    nc = bass.Bass()
    obs4 = nc.declare_dram_parameter("obs4", [32, (T // 4) * BC], F16, isOutput=False)
    wdram = {}
    for name, shape, dt in WEIGHT_SPECS:
        wdram[name] = nc.declare_dram_parameter(name, shape, dt, isOutput=False)
    out = nc.declare_dram_parameter("out", [4, T * BC], F32, isOutput=True)

    with tile.TileContext(nc) as tc:
        with (
            tc.tile_pool(name="singles", bufs=1) as singles,
            tc.tile_pool(name="psum", bufs=1, space="PSUM") as psum,
            tc.tile_pool(name="outp", bufs=3) as outp,
        ):
            wsb = {}
            for name, shape, dt in WEIGHT_SPECS:
                wsb[name] = singles.tile(shape, dt, name=f"w_{name}")
                nc.sync.dma_start(out=wsb[name], in_=wdram[name][:, :])
            obs_sb = singles.tile([32, (T // 4) * BC], F16)
            nc.sync.dma_start(out=obs_sb, in_=obs4[:, :])

            bia = wsb["biases"]

            # obs_enc_all = tanh(wenc.T @ obs4 + enc_b), all steps up front
            oenc = singles.tile([128, (T // 4) * BC], F16)
            NPRE = (T // 4) * BC // 512
            with tc.tile_pool(name="psum_pre", bufs=1, space="PSUM") as psum_pre:
                for i in range(NPRE):
                    ppre = psum_pre.tile([128, 512], F32)
                    nc.tensor.matmul(
                        ppre, wsb["wenc"], obs_sb[:, i * 512 : (i + 1) * 512],
                        start=True, stop=True,
                    )
                    nc.scalar.activation(
                        out=oenc[:, i * 512 : (i + 1) * 512], in_=ppre,
                        func=TANH, bias=bia[:, 5:6],
                    )

            # SB-SB elementwise inputs must share a start partition; outputs
            # are free. fp16 SBUF ops hit the DVE 2x mode.
            NB = 1
            g1 = [singles.tile([64, G], F16, name=f"g1_{g}") for g in range(NB)]
            prod = [singles.tile([64, G], F16, name=f"prod_{g}") for g in range(NB)]
            g2s = [singles.tile([96, G], F16, name=f"g2s_{g}") for g in range(NB)]
            g3t = [singles.tile([64, G], F16, name=f"g3t_{g}") for g in range(NB)]
            ht = [singles.tile([96, G], F16, name=f"ht_{g}") for g in range(NB)]
            icp = [singles.tile([64, G], F16, name=f"icp_{g}") for g in range(NB)]
            omzt = [singles.tile([64, G], F16, name=f"omzt_{g}") for g in range(NB)]
            scr = [singles.tile([64, 2 * G], F16, name=f"scr_{g}") for g in range(NB)]
            hmt = [singles.tile([32, G], F16, name=f"hmt_{g}") for g in range(NB)]
            stkg = [singles.tile([128, G], F16, name=f"stk_{g}") for g in range(1)]
            nc.vector.memset(stkg[0], 0.0)
            nc.vector.memset(stkg[0][96:97, :], 1.0)  # ones row for bias folds

            # PSUM banks, packed so co-resident tensors are never PE-written
            # while another is engine-read concurrently.
            bankA = [psum.tile([128, 128], F32, name=f"bankA{g}") for g in range(1)]
            bankB = [psum.tile([96, 512], F32, name=f"bankB{g}") for g in range(1)]
            p4 = psum.tile([4, 512], F32)

            stk = stkg[0]
            s96 = stk[0:96, :]
            s97 = stk[0:97, :]

            def emit_phi(t):
                # phi matmul for step t, deferred into step t+1's sigma1
                # window so it never blocks the PE FIFO on the chain.
                k = t % 4
                nc.tensor.matmul(
                    p4[0:4, k * BC : (k + 1) * BC],
                    wsb["wphi"][64:96, :], stk[64:96, :],
                    start=True, stop=True,
                )

            def emit_evac(t):
                # evacuate on ScalarE (Identity + phi_b), emitted after
                # sigma1 so it lands in ACT's idle gap, off the DVE FIFO.
                if t < 0 or t % 4 != 3:
                    return
                ch = outp.tile([4, 512], F32)
                nc.scalar.activation(out=ch, in_=p4, func=IDENT, bias=bia[0:4, 6:7])
                nc.sync.dma_start(
                    out=out[0:4, (t // 4) * 512 : (t // 4 + 1) * 512], in_=ch
                )

            for t in range(T):
                c, k = t // 4, t % 4
                b = 0
                p0 = bankA[b][:, 0:G]  # [mult; attn; W_all; oe] rows
                p1 = bankB[b][:, 0:G]
                p2w = bankB[b][0:32, G : 2 * G]
                p2h = bankB[b][0:32, 2 * G : 3 * G]
                p3 = bankB[b][0:64, 3 * G : 4 * G]
                gg1 = g1[b]
                gprod = prod[b]
                gg2 = g2s[b]
                gg3 = g3t[b][32:64, :]
                gh = ht[b][64:96, :]  # matches multI base 64
                gic = icp[b][32:64, :]  # I snapshot, base 32
                gomz = omzt[b][32:64, :]  # base 32, pairs the I snapshot
                gv = scr[b][32:64, 0:G]
                gu = scr[b][32:64, G : 2 * G]
                ghm = hmt[b]  # base 0, pairs z
                oe = oenc[k * 32 : (k + 1) * 32, c * BC : (c + 1) * BC]
                # --- phase A: p0 = [mult_pre; attn_pre; W_all; oe] ---
                if t == 0:
                    nc.tensor.matmul(
                        p0[96:128, :], wsb["id128"][k * 32 : (k + 1) * 32, :], oe,
                        start=True, stop=True, tile_position=(k * 32, 96),
                    )
                nc.tensor.matmul(p0[0:64, :], wsb["wa1"], s96, start=True, stop=True)
                nc.tensor.matmul(p0[64:96, :], wsb["wa2"], s96, start=True, stop=True, tile_position=(0, 64))
                if t > 0:
                    emit_phi(t - 1)  # runs on PE during sigma1
                nc.scalar.activation(out=gg1, in_=p0[0:64, :], func=SIG, bias=bia[0:64, 0:1])
                if t > 0:
                    emit_evac(t - 1)
                nc.vector.tensor_tensor(out=gprod, in0=gg1, in1=p0[64:128, :], op=MULT)
                nc.vector.tensor_copy(out=gic, in_=stk[32:64, :])  # I snapshot
                nc.tensor.matmul(p2w, wsb["id2"], gprod, start=True, stop=True)
                nc.scalar.activation(out=stk[0:32, :], in_=p2w, func=TANH)  # W_new
                # --- phase B ---
                nc.tensor.matmul(p1, wsb["wb"], s96, start=True, stop=True)
                nc.scalar.activation(out=gg2, in_=p1, func=SIG, bias=bia[0:96, 1:2])
                # r*I overwrites the I-slot in place (snapshot taken above)
                # so the h matmul is one contiguous K=96 contraction.
                nc.vector.tensor_tensor(out=stk[32:64, :], in0=gg2[32:64, :], in1=stk[32:64, :], op=MULT)
                nc.tensor.matmul(p2h, wsb["wc"], s96, start=True, stop=True)
                # omz and v hide in the mmC/tanhH window (v reads the snapshot)
                nc.vector.tensor_scalar(
                    out=gomz, in0=gg2[0:32, :], scalar1=-1.0, scalar2=1.0,
                    op0=MULT, op1=ADD,
                )
                nc.vector.tensor_tensor(out=gv, in0=gomz, in1=gic, op=MULT)
                nc.scalar.activation(out=gh, in_=p2h, func=TANH, bias=bia[0:32, 2:3])
                nc.vector.tensor_tensor(out=ghm, in0=gh, in1=gg2[64:96, :], op=MULT)
                nc.vector.tensor_tensor(out=gu, in0=ghm, in1=gg2[0:32, :], op=MULT)
                nc.vector.tensor_tensor(out=stk[32:64, :], in0=gu, in1=gv, op=ADD)  # I_new
                # --- phase C (a_b rides the ones row through wd) ---
                nc.tensor.matmul(p3, wsb["wd"], s97, start=True, stop=True)
                nc.scalar.activation(out=gg3, in_=p3[32:64, :], func=SIG, bias=bia[32:64, 3:4])
                nc.vector.tensor_tensor(out=p3[0:32, :], in0=p3[0:32, :], in1=gg3, op=MULT)
                if t + 1 < T:
                    # next step's obs_enc inject, off the critical chain
                    c2, k2 = (t + 1) // 4, (t + 1) % 4
                    oe2 = oenc[k2 * 32 : (k2 + 1) * 32, c2 * BC : (c2 + 1) * BC]
                    nc.tensor.matmul(
                        p0[96:128, :], wsb["id128"][k2 * 32 : (k2 + 1) * 32, :], oe2,
                        start=True, stop=True, tile_position=(k2 * 32, 96),
                    )
                nc.scalar.activation(out=stk[64:96, :], in_=p3[0:32, :], func=TANH)  # A_new
            emit_phi(T - 1)
            emit_evac(T - 1)
    return nc


_NC_CACHE = None


def kernel(**inputs):
    global _NC_CACHE
    from concourse.bass_utils import run_bass_kernel_spmd

    obs = np.ascontiguousarray(np.asarray(inputs["obs"], np.float32))
    w = _pack_weights({k: v for k, v in inputs.items() if k != "obs"})

    if _NC_CACHE is None:
        _NC_CACHE = _build_nc()
    nc = _NC_CACHE

    in_maps = []
    for i in range(N_CORES):
        m = dict(w)
        m["obs4"] = _pack_obs_shard(obs[:, i * BC : (i + 1) * BC, :])
        in_maps.append(m)

    res = run_bass_kernel_spmd(
        nc, in_maps, core_ids=list(range(N_CORES)), trace=False
    )
    outs = [_unpack_out(np.asarray(res.results[i]["out"])) for i in range(N_CORES)]
    return np.concatenate(outs, axis=1).astype(np.float32)  # [T, B, O]
